# revision 38
# baseline (speedup 1.0000x reference)
"""Trainium2 Bass kernel for nn_BasicBlock_HMU (two HMU layers + sync BN + residual).

Sharding: data-parallel over batch (8 cores x 512 rows); params replicated.
BN batch statistics are AllGathered + reduced on-chip (sync BN).

Design — transposed GEMM orientation (units n on partitions, batch on the
free axis), bf16 operands, fp32 PSUM accumulation:

  quad_tile[n_blk, b] = lam*|x-mu|^2 + sum_k (v_k.x - v_k.mu)^2

* mu-part:  8 K-chunk matmuls + 1 K=2 ext matmul ([sq_hi;sq_lo] rows x
  [lam;lam]) for the lam*|x|^2 rank-1 term.  The per-n consts lam*|mu|^2
  are dropped entirely — any per-unit constant added to quad multiplies
  (1+z) by a per-unit factor, which BatchNorm absorbs exactly.
* v-part: 8 K-chunk matmuls per k-plane; the -v_k.mu constant is applied
  as the per-partition BIAS of the Square eviction (f32, free).
* BN batch stats = free-axis tensor_reduce on DVE (no stats matmuls);
  BN affine params are per-partition [128,1] APs (no broadcasts).
* L1 output (hT, bf16, [n,b]) IS the layer-2 moving operand (no transpose).
* W2 (bf16, 80KB/partition) is fully prefetched into SBUF during L1.
* one 8KB AllGather per layer; only its latency is exposed.
* output leaves via DMA-transpose (16-bit xbar path) + residual add; the
  PE does zero transposes.
"""

import numpy as np
import ml_dtypes

import concourse.bacc as bacc
import concourse.mybir as mybir
import concourse.tile as tile

try:
    from concourse.bass_utils import run_bass_kernel_spmd
except ImportError:  # pragma: no cover
    from bass_utils import run_bass_kernel_spmd

F32 = mybir.dt.float32
BF16 = mybir.dt.bfloat16
Alu = mybir.AluOpType
Act = mybir.ActivationFunctionType

N_CORES = 8
B, D, N, K = 4096, 1024, 1024, 4
BS = B // N_CORES          # 512 rows per core
CH = D // 128              # 8 contraction chunks
NBLK = N // 128            # 8 unit blocks (128 units each)
NG = 1 + K                 # 5 groups per block: mu + 4 v-planes
BCOLS = NG * 128           # 640 W cols per block
WTC = NBLK * CH * BCOLS    # 40960 tiled W cols
BN_EPS = 1e-5

_CACHE = {}


def _build_nc(reps=1, collectives=True):
    nc = bacc.Bacc("TRN2", target_bir_lowering=False, debug=False,
                   num_devices=N_CORES)

    # --- dram inputs (host pre-tiled; see _host_prep) ---
    xtb = nc.dram_tensor("xtb", [128, CH * BS], BF16, kind="ExternalInput").ap()
    xn_s = nc.dram_tensor("xn_s", [BS, N], F32, kind="ExternalInput").ap()
    e1_s = nc.dram_tensor("e1_s", [2, BS], BF16, kind="ExternalInput").ap()
    W1t = nc.dram_tensor("W1t", [128, WTC], BF16, kind="ExternalInput").ap()
    W2t = nc.dram_tensor("W2t", [128, WTC], BF16, kind="ExternalInput").ap()
    wel = nc.dram_tensor("wel", [2, 2 * N], BF16, kind="ExternalInput").ap()
    cvt = nc.dram_tensor("cvt", [128, 2 * 4 * NBLK], F32,
                         kind="ExternalInput").ap()
    gbt = nc.dram_tensor("gbt", [128, 4 * NBLK], F32, kind="ExternalInput").ap()
    onescol = nc.dram_tensor("onescol", [128, 1], BF16,
                             kind="ExternalInput").ap()
    out = nc.dram_tensor("out", [BS, N], F32, kind="ExternalOutput").ap()

    from contextlib import ExitStack
    with tile.TileContext(nc) as tc:
        with ExitStack() as stack:
            pool = lambda *a, **kw: stack.enter_context(tc.tile_pool(*a, **kw))
            constp = pool(name="const", bufs=1)
            w2p = pool(name="w2res", bufs=1)
            w1p = pool(name="w1s", bufs=2)
            qp = pool(name="qp", bufs=3)
            zp = pool(name="zp", bufs=1)
            sqp = pool(name="sqp", bufs=5)
            htp = pool(name="htp", bufs=1)
            hsqp = pool(name="hsqp", bufs=2)
            otp = pool(name="otp", bufs=8)
            onatp = pool(name="onatp", bufs=4)
            statp = pool(name="statp", bufs=2)
            gathp = pool(name="gathp", bufs=2)
            finp = pool(name="finp", bufs=6)
            stp = pool(name="stp", bufs=2)
            xnp = pool(name="xnp", bufs=4)
            outp = pool(name="outp", bufs=2)
            e2p = pool(name="e2p", bufs=1)
            pmm = pool(name="pmm", bufs=6, space="PSUM")
            pst = pool(name="pst", bufs=1, space="PSUM")
            dramp = pool(name="dram", bufs=2, space="DRAM")

            # ---- small resident inputs (Pool-engine SWDGE queue; the sync
            # queue is kept free for xt + the W streams) ----
            e1t = constp.tile([2, BS], BF16)
            nc.gpsimd.dma_start(e1t[:], e1_s)
            welt = constp.tile([2, 2 * N], BF16)
            nc.gpsimd.dma_start(welt[:], wel)
            cvtt = constp.tile([128, 2 * 4 * NBLK], F32)
            nc.gpsimd.dma_start(cvtt[:], cvt)
            gb = constp.tile([128, 4 * NBLK], F32)
            nc.gpsimd.dma_start(gb[:], gbt)
            onec = constp.tile([128, 1], BF16)
            nc.gpsimd.dma_start(onec[:], onescol)
            epsc = constp.tile([128, 1], F32)
            nc.gpsimd.memset(epsc[:], BN_EPS)

            def body():
                xt = constp.tile([128, CH * BS], BF16, tag="xt")
                nc.sync.dma_start(xt[:], xtb)
                w2r = w2p.tile([128, WTC], BF16, tag="w2r")
                hT = htp.tile([128, CH * BS], BF16, tag="hT")
                e2t = e2p.tile([2, BS], BF16, tag="e2t")

                for L in range(2):
                    lhs = (xt, hT)[L]
                    ext = (e1t, e2t)[L]
                    z = zp.tile([128, NBLK * BS], F32, tag="z")
                    stats = statp.tile([128, 2 * NBLK], F32, tag=f"stats{L}")
                    if L == 1:
                        # preload x (natural, f32) for the residual while the
                        # DMA device is otherwise idle (W2 already resident)
                        xnts = []
                        for j in range(BS // 128):
                            xnt = xnp.tile([128, N], F32, tag="xn")
                            nc.scalar.dma_start(
                                xnt[:], xn_s[j * 128:(j + 1) * 128, :])
                            xnts.append(xnt)

                    # ---- GEMM blocks ----
                    for c in range(NBLK):
                        if L == 0:
                            if c == 0:
                                w1 = []
                                for cc in range(2):
                                    wt = w1p.tile([128, CH * BCOLS], BF16,
                                                  tag="w1")
                                    nc.sync.dma_start(
                                        wt[:],
                                        W1t[:, cc * CH * BCOLS:
                                            (cc + 1) * CH * BCOLS])
                                    w1.append(wt)
                                wt = w1[0]
                            else:
                                wt = w1[c % 2]
                            if c + 2 <= NBLK - 1:
                                wtn = w1p.tile([128, CH * BCOLS], BF16,
                                               tag="w1")
                                nc.sync.dma_start(
                                    wtn[:],
                                    W1t[:, (c + 2) * CH * BCOLS:
                                        (c + 3) * CH * BCOLS])
                                w1[(c + 2) % 2] = wtn
                            # interleave W2 prefetch behind W1 blocks
                            nc.sync.dma_start(
                                w2r[:, c * CH * BCOLS:(c + 1) * CH * BCOLS],
                                W2t[:, c * CH * BCOLS:(c + 1) * CH * BCOLS])
                            wsl = lambda g, ch: wt[:, ch * BCOLS + g * 128:
                                                   ch * BCOLS + (g + 1) * 128]
                        else:
                            wsl = lambda g, ch, c=c: w2r[
                                :, (c * CH + ch) * BCOLS + g * 128:
                                   (c * CH + ch) * BCOLS + (g + 1) * 128]

                        pms = []
                        for g in range(NG):
                            pm = pmm.tile([128, BS], F32, tag="pm")
                            for ch in range(CH):
                                nc.tensor.matmul(
                                    pm[:], wsl(g, ch),
                                    lhs[:, ch * BS:(ch + 1) * BS],
                                    start=(ch == 0),
                                    stop=(ch == CH - 1 and g > 0))
                            if g == 0:
                                # rank-1 lam*|x|^2 via [sq_hi;sq_lo] ext rows
                                nc.tensor.matmul(
                                    pm[:],
                                    welt[:, L * N + c * 128:
                                         L * N + (c + 1) * 128],
                                    ext[:], start=False, stop=True)
                            pms.append(pm)

                        # ---- eviction: q = mu + sum_k (proj + cv_k)^2 ----
                        sqs = []
                        for g in range(1, NG):
                            sq = sqp.tile([128, BS], F32, tag="sq")
                            nc.scalar.activation(
                                sq[:], pms[g][:], Act.Square,
                                bias=cvtt[:, L * 32 + c * 4 + g - 1:
                                          L * 32 + c * 4 + g])
                            sqs.append(sq)
                        q = qp.tile([128, BS], F32, tag="q")
                        t2 = qp.tile([128, BS], F32, tag="q")
                        nc.vector.tensor_tensor(
                            out=q[:], in0=pms[0][:], in1=sqs[0][:], op=Alu.add)
                        nc.gpsimd.tensor_tensor(
                            out=t2[:], in0=sqs[1][:], in1=sqs[2][:], op=Alu.add)
                        nc.vector.tensor_tensor(
                            out=q[:], in0=q[:], in1=sqs[3][:], op=Alu.add)
                        nc.gpsimd.tensor_tensor(
                            out=q[:], in0=q[:], in1=t2[:], op=Alu.add)
                        # y = exp(-q/D) in place, then z = y-1, S1, S2
                        nc.scalar.activation(q[:], q[:], Act.Exp,
                                             scale=-1.0 / D)
                        nc.vector.tensor_scalar(
                            out=z[:, c * BS:(c + 1) * BS], in0=q[:],
                            scalar1=-1.0, scalar2=None, op0=Alu.add)
                        nc.vector.tensor_reduce(
                            out=stats[:, c:c + 1],
                            in_=z[:, c * BS:(c + 1) * BS],
                            axis=mybir.AxisListType.X, op=Alu.add)
                        zsq = sqp.tile([128, BS], F32, tag="sq")
                        nc.vector.tensor_tensor(
                            out=zsq[:], in0=z[:, c * BS:(c + 1) * BS],
                            in1=z[:, c * BS:(c + 1) * BS], op=Alu.mult)
                        nc.vector.tensor_reduce(
                            out=stats[:, NBLK + c:NBLK + c + 1], in_=zsq[:],
                            axis=mybir.AxisListType.X, op=Alu.add)

                    # ---- sync BN: AllGather + on-chip reduce ----
                    cin = dramp.tile([128, 2 * NBLK], F32, tag=f"cin{L}")
                    cout = dramp.tile([N_CORES * 128, 2 * NBLK], F32,
                                      tag=f"cout{L}", addr_space="Shared")
                    nc.sync.dma_start(cin[:], stats[:])
                    if collectives:
                        nc.gpsimd.collective_compute(
                            "AllGather", Alu.bypass,
                            replica_groups=[list(range(N_CORES))],
                            ins=[cin[:].opt()], outs=[cout[:].opt()])
                    else:
                        nc.sync.dma_start(cout[0:128, :], cin[:])
                    gath = gathp.tile([128, N_CORES * 2 * NBLK], F32,
                                      tag="gath")
                    nc.sync.dma_start(
                        gath[:].rearrange("p (c s) -> p c s", c=N_CORES),
                        cout[:].rearrange("(c p) s -> p c s", c=N_CORES))
                    red = finp.tile([128, 2 * NBLK], F32, tag="fin")
                    nc.vector.tensor_tensor(
                        out=red[:], in0=gath[:, 0:16], in1=gath[:, 16:32],
                        op=Alu.add)
                    for cc in range(2, N_CORES):
                        eng = nc.vector if cc % 2 == 0 else nc.gpsimd
                        eng.tensor_tensor(
                            out=red[:], in0=red[:],
                            in1=gath[:, cc * 16:(cc + 1) * 16], op=Alu.add)

                    # ---- finalize: s = g/sqrt(var+eps), t = beta - m*s ----
                    st = stp.tile([128, 2 * NBLK], F32, tag=f"st{L}")
                    m = finp.tile([128, NBLK], F32, tag="fin")
                    msq = finp.tile([128, NBLK], F32, tag="fin")
                    sd = finp.tile([128, NBLK], F32, tag="fin")
                    rs = finp.tile([128, NBLK], F32, tag="fin")
                    nc.vector.tensor_scalar(
                        out=m[:], in0=red[:, 0:NBLK], scalar1=1.0 / B,
                        scalar2=None, op0=Alu.mult)
                    nc.vector.tensor_tensor(out=msq[:], in0=m[:], in1=m[:],
                                            op=Alu.mult)
                    nc.vector.scalar_tensor_tensor(
                        out=sd[:], in0=red[:, NBLK:2 * NBLK], scalar=1.0 / B,
                        in1=msq[:], op0=Alu.mult, op1=Alu.subtract)
                    nc.scalar.activation(sd[:], sd[:], Act.Sqrt, bias=epsc[:])
                    nc.vector.reciprocal(rs[:], sd[:])
                    g_cols = gb[:, 2 * NBLK * L:2 * NBLK * L + NBLK]
                    b_cols = gb[:, 2 * NBLK * L + NBLK:2 * NBLK * (L + 1)]
                    nc.vector.tensor_tensor(
                        out=st[:, 0:NBLK], in0=rs[:], in1=g_cols, op=Alu.mult)
                    nc.vector.tensor_tensor(
                        out=m[:], in0=m[:], in1=st[:, 0:NBLK], op=Alu.mult)
                    nc.vector.scalar_tensor_tensor(
                        out=st[:, NBLK:2 * NBLK], in0=m[:], scalar=-1.0,
                        in1=b_cols, op0=Alu.mult, op1=Alu.add)

                    if L == 0:
                        # ---- h = s*z + t  (bf16, [n,b] layout = L2 moving) ----
                        hsp = pst.tile([1, BS], F32, tag="hsp")
                        for c in range(NBLK):
                            if c % 3 == 0:
                                nc.scalar.activation(
                                    hT[:, c * BS:(c + 1) * BS],
                                    z[:, c * BS:(c + 1) * BS], Act.Identity,
                                    scale=st[:, c:c + 1],
                                    bias=st[:, NBLK + c:NBLK + c + 1])
                            else:
                                eng = (nc.vector, nc.gpsimd)[c % 2]
                                eng.tensor_scalar(
                                    out=hT[:, c * BS:(c + 1) * BS],
                                    in0=z[:, c * BS:(c + 1) * BS],
                                    scalar1=st[:, c:c + 1],
                                    scalar2=st[:, NBLK + c:NBLK + c + 1],
                                    op0=Alu.mult, op1=Alu.add)
                            hsq = hsqp.tile([128, BS], BF16, tag="hsq")
                            eng2 = (nc.gpsimd, nc.vector)[c % 2]
                            eng2.tensor_tensor(
                                out=hsq[:], in0=hT[:, c * BS:(c + 1) * BS],
                                in1=hT[:, c * BS:(c + 1) * BS], op=Alu.mult)
                            nc.tensor.matmul(hsp[:], onec[:], hsq[:],
                                             start=(c == 0),
                                             stop=(c == NBLK - 1))
                        # e2 rows: [hsq_hi; hsq_lo] (bf16 hi/lo split)
                        hhi = hsqp.tile([1, BS], BF16, tag="hsq")
                        hlo = hsqp.tile([1, BS], BF16, tag="hsq")
                        nc.scalar.copy(hhi[:], hsp[:])
                        nc.vector.tensor_tensor(
                            out=hlo[:], in0=hsp[:], in1=hhi[:],
                            op=Alu.subtract)
                        nc.sync.dma_start(e2t[0:1, :], hhi[:])
                        nc.sync.dma_start(e2t[1:2, :], hlo[:])
                    else:
                        # ---- out^T = s*z + t (bf16), DMA-transpose, +x ----
                        ots = []
                        for c in range(NBLK):
                            ot = otp.tile([128, BS], BF16, tag="ot")
                            eng = (nc.vector, nc.gpsimd)[c % 2]
                            eng.tensor_scalar(
                                out=ot[:], in0=z[:, c * BS:(c + 1) * BS],
                                scalar1=st[:, c:c + 1],
                                scalar2=st[:, NBLK + c:NBLK + c + 1],
                                op0=Alu.mult, op1=Alu.add)
                            ots.append(ot)
                        onats = []
                        for j in range(BS // 128):
                            onat = onatp.tile([128, N], BF16, tag="onat")
                            for c in range(NBLK):
                                nc.scalar.dma_start(
                                    onat[:, c * 128:(c + 1) * 128],
                                    ots[c][:, j * 128:(j + 1) * 128],
                                    transpose=True)
                            onats.append(onat)
                        for j in range(BS // 128):
                            on = outp.tile([128, N], F32, tag="on")
                            eng = (nc.vector, nc.gpsimd)[j % 2]
                            eng.tensor_tensor(out=on[:], in0=onats[j][:],
                                              in1=xnts[j][:], op=Alu.add)
                            nc.sync.dma_start(
                                out[j * 128:(j + 1) * 128, :], on[:])

            for _rep in range(reps):
                body()

    nc.compile()
    return nc


def _host_prep(x, mu1, lam1, v1, g1, b1, mu2, lam2, v2, g2, b2):
    """Build device-input arrays: bf16 pre-tiled weights, per-core x slices."""
    bf16 = ml_dtypes.bfloat16

    def build_w(mu, lam_, v):
        mu64 = mu.astype(np.float64)
        v64 = v.astype(np.float64)
        lam64 = lam_.astype(np.float64)
        Wcols = np.empty((D, NBLK * BCOLS), np.float32)   # weight rows
        wmu = (-2.0 * lam64[:, None] * mu64).T            # (d, n)
        cv = -(v64 * mu64[:, None, :]).sum(-1)            # (n, k)
        for c in range(NBLK):
            ns = slice(c * 128, (c + 1) * 128)
            base = c * BCOLS
            Wcols[:, base:base + 128] = wmu[:, ns]
            for k in range(K):
                b0 = base + (k + 1) * 128
                Wcols[:, b0:b0 + 128] = v[ns, k, :].T
        # tile to SBUF layout: [128, (c ch) * 640]
        Wt = np.empty((128, WTC), np.float32)
        for c in range(NBLK):
            for ch in range(CH):
                Wt[:, (c * CH + ch) * BCOLS:(c * CH + ch + 1) * BCOLS] = \
                    Wcols[ch * 128:(ch + 1) * 128, c * BCOLS:(c + 1) * BCOLS]
        # cv as per-partition f32 columns: [128, 4*NBLK], col = c*4+k
        cvc = cv.reshape(NBLK, 128, K).transpose(1, 0, 2).reshape(128, -1)
        return Wt.astype(bf16), cvc.astype(np.float32)

    W1T, cv1 = build_w(mu1, lam1, v1)
    W2T, cv2 = build_w(mu2, lam2, v2)
    wel_a = np.empty((2, 2 * N), np.float32)
    wel_a[0, 0:N] = lam1
    wel_a[1, 0:N] = lam1
    wel_a[0, N:] = lam2
    wel_a[1, N:] = lam2
    cvt_a = np.concatenate([cv1, cv2], axis=1)
    xT = x.T.astype(np.float32)                           # (d, b)
    xsq = (x.astype(np.float64) ** 2).sum(1).astype(np.float32)
    gbt = np.stack([g1, b1, g2, b2]) \
        .reshape(4, NBLK, 128).transpose(2, 0, 1).reshape(128, 4 * NBLK) \
        .astype(np.float32)
    onesc = np.ones((128, 1), bf16)

    in_maps = []
    for core in range(N_CORES):
        rs = slice(core * BS, (core + 1) * BS)
        xTs = xT[:, rs]                                   # (1024, 512)
        xtt = np.empty((128, CH * BS), np.float32)
        for ch in range(CH):
            xtt[:, ch * BS:(ch + 1) * BS] = xTs[ch * 128:(ch + 1) * 128, :]
        sq = xsq[rs].astype(np.float32)
        sq_hi = sq.astype(bf16)
        sq_lo = (sq.astype(np.float64)
                 - sq_hi.astype(np.float64)).astype(bf16)
        e1 = np.empty((2, BS), bf16)
        e1[0] = sq_hi
        e1[1] = sq_lo
        in_maps.append({
            "xtb": xtt.astype(bf16),
            "xn_s": np.ascontiguousarray(x[rs]).astype(np.float32),
            "e1_s": e1,
            "W1t": W1T, "W2t": W2T,
            "wel": wel_a.astype(bf16), "cvt": cvt_a,
            "gbt": gbt, "onescol": onesc,
        })
    return in_maps


def kernel(x, mu1, lam1, v1, g1, b1, mu2, lam2, v2, g2, b2):
    if "nc" not in _CACHE:
        _CACHE["nc"] = _build_nc()
    nc = _CACHE["nc"]
    in_maps = _host_prep(x, mu1, lam1, v1, g1, b1, mu2, lam2, v2, g2, b2)
    res = run_bass_kernel_spmd(nc, in_maps, list(range(N_CORES)))
    return np.concatenate([res.results[c]["out"] for c in range(N_CORES)], axis=0)


# revision 45
# speedup vs baseline: 2.3158x; 2.3158x over previous
"""Trainium2 Bass kernel for nn_BasicBlock_HMU (two HMU layers + sync BN + residual).

Sharding: data-parallel over batch (8 cores x 512 rows); params replicated.
BN batch statistics are AllGathered + reduced on-chip (sync BN).

Design — transposed GEMM orientation (units n on partitions, batch on the
free axis), bf16 operands, fp32 PSUM accumulation:

  quad_tile[n_blk, b] = lam*|x-mu|^2 + sum_k (v_k.x - v_k.mu)^2

* mu-part:  8 K-chunk matmuls + 1 K=2 ext matmul ([sq_hi;sq_lo] rows x
  [lam;lam]) for the lam*|x|^2 rank-1 term.  The per-n consts lam*|mu|^2
  are dropped entirely — any per-unit constant added to quad multiplies
  (1+z) by a per-unit factor, which BatchNorm absorbs exactly.
* v-part: 8 K-chunk matmuls per k-plane; the -v_k.mu constant is applied
  as the per-partition BIAS of the Square eviction (f32, free).
* BN batch stats = free-axis tensor_reduce on DVE (no stats matmuls);
  BN affine params are per-partition [128,1] APs (no broadcasts).
* L1 output (hT, bf16, [n,b]) IS the layer-2 moving operand (no transpose).
* W2 (bf16, 80KB/partition) is fully prefetched into SBUF during L1.
* one 8KB AllGather per layer; only its latency is exposed.
* output leaves via DMA-transpose (16-bit xbar path) + residual add; the
  PE does zero transposes.
"""

import numpy as np
import ml_dtypes

import concourse.bacc as bacc
import concourse.mybir as mybir
import concourse.tile as tile

try:
    from concourse.bass_utils import run_bass_kernel_spmd
except ImportError:  # pragma: no cover
    from bass_utils import run_bass_kernel_spmd

F32 = mybir.dt.float32
BF16 = mybir.dt.bfloat16
Alu = mybir.AluOpType
Act = mybir.ActivationFunctionType

N_CORES = 8
B, D, N, K = 4096, 1024, 1024, 4
BS = B // N_CORES          # 512 rows per core
CH = D // 128              # 8 contraction chunks
NBLK = N // 128            # 8 unit blocks (128 units each)
NG = 1 + K                 # 5 groups per block: mu + 4 v-planes
BCOLS = NG * 128           # 640 W cols per block
WTC = NBLK * CH * BCOLS    # 40960 tiled W cols
BN_EPS = 1e-5

_CACHE = {}


def _build_nc(reps=1, collectives=True):
    nc = bacc.Bacc("TRN2", target_bir_lowering=False, debug=False,
                   num_devices=N_CORES)

    # --- dram inputs (host pre-tiled; see _host_prep) ---
    xtb = nc.dram_tensor("xtb", [128, CH * BS], BF16, kind="ExternalInput").ap()
    xn_s = nc.dram_tensor("xn_s", [BS, N], F32, kind="ExternalInput").ap()
    e1_s = nc.dram_tensor("e1_s", [2, BS], BF16, kind="ExternalInput").ap()
    W1t = nc.dram_tensor("W1t", [128, WTC], BF16, kind="ExternalInput").ap()
    W2t = nc.dram_tensor("W2t", [128, WTC], BF16, kind="ExternalInput").ap()
    wel = nc.dram_tensor("wel", [2, 2 * N], BF16, kind="ExternalInput").ap()
    cvt = nc.dram_tensor("cvt", [128, 2 * 4 * NBLK], F32,
                         kind="ExternalInput").ap()
    gbt = nc.dram_tensor("gbt", [128, 4 * NBLK], F32, kind="ExternalInput").ap()
    onescol = nc.dram_tensor("onescol", [128, 1], BF16,
                             kind="ExternalInput").ap()
    out = nc.dram_tensor("out", [BS, N], F32, kind="ExternalOutput").ap()

    from contextlib import ExitStack
    with tile.TileContext(nc) as tc:
        with ExitStack() as stack:
            pool = lambda *a, **kw: stack.enter_context(tc.tile_pool(*a, **kw))
            constp = pool(name="const", bufs=1)
            w2p = pool(name="w2res", bufs=1)
            w1p = pool(name="w1s", bufs=2)
            qp = pool(name="qp", bufs=3)
            zp = pool(name="zp", bufs=1)
            sqp = pool(name="sqp", bufs=5)
            htp = pool(name="htp", bufs=1)
            hsqp = pool(name="hsqp", bufs=2)
            otp = pool(name="otp", bufs=8)
            onatp = pool(name="onatp", bufs=4)
            statp = pool(name="statp", bufs=2)
            gathp = pool(name="gathp", bufs=2)
            finp = pool(name="finp", bufs=6)
            stp = pool(name="stp", bufs=2)
            xnp = pool(name="xnp", bufs=4)
            outp = pool(name="outp", bufs=4)
            e2p = pool(name="e2p", bufs=1)
            pmm = pool(name="pmm", bufs=6, space="PSUM")
            pst = pool(name="pst", bufs=1, space="PSUM")
            dramp = pool(name="dram", bufs=2, space="DRAM")

            # ---- small resident inputs (Pool-engine SWDGE queue; the sync
            # queue is kept free for xt + the W streams) ----
            e1t = constp.tile([2, BS], BF16)
            nc.gpsimd.dma_start(e1t[:], e1_s)
            welt = constp.tile([2, 2 * N], BF16)
            nc.gpsimd.dma_start(welt[:], wel)
            cvtt = constp.tile([128, 2 * 4 * NBLK], F32)
            nc.gpsimd.dma_start(cvtt[:], cvt)
            gb = constp.tile([128, 4 * NBLK], F32)
            nc.gpsimd.dma_start(gb[:], gbt)
            onec = constp.tile([128, 1], BF16)
            nc.gpsimd.dma_start(onec[:], onescol)
            epsc = constp.tile([128, 1], F32)
            nc.gpsimd.memset(epsc[:], BN_EPS)

            def body():
                xt = constp.tile([128, CH * BS], BF16, tag="xt")
                nc.sync.dma_start(xt[:, 0:2 * BS], xtb[:, 0:2 * BS])
                nc.sync.dma_start(xt[:, 2 * BS:], xtb[:, 2 * BS:])
                w2r = w2p.tile([128, WTC], BF16, tag="w2r")
                hT = htp.tile([128, CH * BS], BF16, tag="hT")
                e2t = e2p.tile([2, BS], BF16, tag="e2t")

                for L in range(2):
                    lhs = (xt, hT)[L]
                    ext = (e1t, e2t)[L]
                    z = zp.tile([128, NBLK * BS], F32, tag="z")
                    stats = statp.tile([128, 2 * NBLK], F32, tag=f"stats{L}")
                    if L == 1:
                        # preload x (natural, f32) for the residual while the
                        # DMA device is otherwise idle (W2 already resident)
                        xnts = []
                        for j in range(BS // 128):
                            xnt = xnp.tile([128, N], F32, tag="xn")
                            nc.scalar.dma_start(
                                xnt[:], xn_s[j * 128:(j + 1) * 128, :])
                            xnts.append(xnt)

                    # ---- GEMM blocks ----
                    for c in range(NBLK):
                        if L == 0:
                            if c == 0:
                                w1 = []
                                for cc in range(2):
                                    wt = w1p.tile([128, CH * BCOLS], BF16,
                                                  tag="w1")
                                    nc.sync.dma_start(
                                        wt[:],
                                        W1t[:, cc * CH * BCOLS:
                                            (cc + 1) * CH * BCOLS])
                                    w1.append(wt)
                                wt = w1[0]
                            else:
                                wt = w1[c % 2]
                            if c + 2 <= NBLK - 1:
                                wtn = w1p.tile([128, CH * BCOLS], BF16,
                                               tag="w1")
                                nc.sync.dma_start(
                                    wtn[:],
                                    W1t[:, (c + 2) * CH * BCOLS:
                                        (c + 3) * CH * BCOLS])
                                w1[(c + 2) % 2] = wtn
                            # interleave W2 prefetch behind W1 blocks
                            nc.sync.dma_start(
                                w2r[:, c * CH * BCOLS:(c + 1) * CH * BCOLS],
                                W2t[:, c * CH * BCOLS:(c + 1) * CH * BCOLS])
                            wsl = lambda g, ch: wt[:, ch * BCOLS + g * 128:
                                                   ch * BCOLS + (g + 1) * 128]
                        else:
                            wsl = lambda g, ch, c=c: w2r[
                                :, (c * CH + ch) * BCOLS + g * 128:
                                   (c * CH + ch) * BCOLS + (g + 1) * 128]

                        # v-plane groups first, mu last: the Squares and the
                        # sq tree run under the mu matmuls, so only one DVE
                        # add + exp + stats trail the block's last matmul.
                        sqs = []
                        for g in range(1, NG):
                            pm = pmm.tile([128, BS], F32, tag="pm")
                            for ch in range(CH):
                                nc.tensor.matmul(
                                    pm[:], wsl(g, ch),
                                    lhs[:, ch * BS:(ch + 1) * BS],
                                    start=(ch == 0), stop=(ch == CH - 1))
                            sq = sqp.tile([128, BS], F32, tag="sq")
                            nc.scalar.activation(
                                sq[:], pm[:], Act.Square,
                                bias=cvtt[:, L * 32 + c * 4 + g - 1:
                                          L * 32 + c * 4 + g])
                            sqs.append(sq)
                        mu_pm = pmm.tile([128, BS], F32, tag="pm")
                        for ch in range(CH):
                            nc.tensor.matmul(
                                mu_pm[:], wsl(0, ch),
                                lhs[:, ch * BS:(ch + 1) * BS],
                                start=(ch == 0), stop=False)
                        # rank-1 lam*|x|^2 via [sq_hi;sq_lo] ext rows
                        nc.tensor.matmul(
                            mu_pm[:],
                            welt[:, L * N + c * 128:L * N + (c + 1) * 128],
                            ext[:], start=False, stop=True)

                        qa = qp.tile([128, BS], F32, tag="q")
                        t2 = qp.tile([128, BS], F32, tag="q")
                        q = qp.tile([128, BS], F32, tag="q")
                        nc.vector.tensor_tensor(
                            out=qa[:], in0=sqs[0][:], in1=sqs[1][:], op=Alu.add)
                        nc.gpsimd.tensor_tensor(
                            out=t2[:], in0=sqs[2][:], in1=sqs[3][:], op=Alu.add)
                        nc.gpsimd.tensor_tensor(
                            out=qa[:], in0=qa[:], in1=t2[:], op=Alu.add)
                        nc.vector.tensor_tensor(
                            out=q[:], in0=qa[:], in1=mu_pm[:], op=Alu.add)
                        # y = exp(-q/D) in place, then z = y-1, S1, S2
                        nc.scalar.activation(q[:], q[:], Act.Exp,
                                             scale=-1.0 / D)
                        nc.vector.tensor_scalar(
                            out=z[:, c * BS:(c + 1) * BS], in0=q[:],
                            scalar1=-1.0, scalar2=None, op0=Alu.add)
                        nc.vector.tensor_reduce(
                            out=stats[:, c:c + 1],
                            in_=z[:, c * BS:(c + 1) * BS],
                            axis=mybir.AxisListType.X, op=Alu.add)
                        zsq = sqp.tile([128, BS], F32, tag="sq")
                        nc.gpsimd.tensor_tensor(
                            out=zsq[:], in0=z[:, c * BS:(c + 1) * BS],
                            in1=z[:, c * BS:(c + 1) * BS], op=Alu.mult)
                        nc.vector.tensor_reduce(
                            out=stats[:, NBLK + c:NBLK + c + 1], in_=zsq[:],
                            axis=mybir.AxisListType.X, op=Alu.add)

                    # ---- sync BN: AllGather + on-chip reduce ----
                    cin = dramp.tile([128, 2 * NBLK], F32, tag=f"cin{L}")
                    cout = dramp.tile([N_CORES * 128, 2 * NBLK], F32,
                                      tag=f"cout{L}", addr_space="Shared")
                    nc.sync.dma_start(cin[:], stats[:])
                    # prewarm the Sqrt act table while the collective runs
                    warm = finp.tile([128, 1], F32, tag="fin")
                    nc.scalar.activation(warm[:], epsc[:], Act.Sqrt)
                    if collectives:
                        nc.gpsimd.collective_compute(
                            "AllGather", Alu.bypass,
                            replica_groups=[list(range(N_CORES))],
                            ins=[cin[:].opt()], outs=[cout[:].opt()])
                    else:
                        nc.sync.dma_start(cout[0:128, :], cin[:])
                    gath = gathp.tile([128, N_CORES * 2 * NBLK], F32,
                                      tag="gath")
                    nc.sync.dma_start(
                        gath[:].rearrange("p (c s) -> p c s", c=N_CORES),
                        cout[:].rearrange("(c p) s -> p c s", c=N_CORES))
                    # tree-reduce the 8 gathered core contributions
                    red = finp.tile([128, 2 * NBLK], F32, tag="fin")
                    h0 = finp.tile([128, 2 * NBLK], F32, tag="fin")
                    h1 = finp.tile([128, 2 * NBLK], F32, tag="fin")
                    h2 = finp.tile([128, 2 * NBLK], F32, tag="fin")
                    gs = lambda i: gath[:, i * 16:(i + 1) * 16]
                    nc.vector.tensor_tensor(out=red[:], in0=gs(0), in1=gs(1),
                                            op=Alu.add)
                    nc.gpsimd.tensor_tensor(out=h0[:], in0=gs(2), in1=gs(3),
                                            op=Alu.add)
                    nc.vector.tensor_tensor(out=h1[:], in0=gs(4), in1=gs(5),
                                            op=Alu.add)
                    nc.gpsimd.tensor_tensor(out=h2[:], in0=gs(6), in1=gs(7),
                                            op=Alu.add)
                    nc.vector.tensor_tensor(out=red[:], in0=red[:], in1=h0[:],
                                            op=Alu.add)
                    nc.gpsimd.tensor_tensor(out=h1[:], in0=h1[:], in1=h2[:],
                                            op=Alu.add)
                    nc.vector.tensor_tensor(out=red[:], in0=red[:], in1=h1[:],
                                            op=Alu.add)

                    # ---- finalize: s = g/sqrt(var+eps), t = beta - m*s ----
                    st = stp.tile([128, 2 * NBLK], F32, tag=f"st{L}")
                    m = finp.tile([128, NBLK], F32, tag="fin")
                    msq = finp.tile([128, NBLK], F32, tag="fin")
                    sd = finp.tile([128, NBLK], F32, tag="fin")
                    rs = finp.tile([128, NBLK], F32, tag="fin")
                    nc.vector.tensor_scalar(
                        out=m[:], in0=red[:, 0:NBLK], scalar1=1.0 / B,
                        scalar2=None, op0=Alu.mult)
                    nc.vector.tensor_tensor(out=msq[:], in0=m[:], in1=m[:],
                                            op=Alu.mult)
                    nc.vector.scalar_tensor_tensor(
                        out=sd[:], in0=red[:, NBLK:2 * NBLK], scalar=1.0 / B,
                        in1=msq[:], op0=Alu.mult, op1=Alu.subtract)
                    nc.scalar.activation(sd[:], sd[:], Act.Sqrt, bias=epsc[:])
                    nc.vector.reciprocal(rs[:], sd[:])
                    g_cols = gb[:, 2 * NBLK * L:2 * NBLK * L + NBLK]
                    b_cols = gb[:, 2 * NBLK * L + NBLK:2 * NBLK * (L + 1)]
                    nc.vector.tensor_tensor(
                        out=st[:, 0:NBLK], in0=rs[:], in1=g_cols, op=Alu.mult)
                    nc.vector.tensor_tensor(
                        out=m[:], in0=m[:], in1=st[:, 0:NBLK], op=Alu.mult)
                    nc.vector.scalar_tensor_tensor(
                        out=st[:, NBLK:2 * NBLK], in0=m[:], scalar=-1.0,
                        in1=b_cols, op0=Alu.mult, op1=Alu.add)

                    if L == 0:
                        # ---- h = s*z + t  (bf16, [n,b] layout = L2 moving) ----
                        hsp = pst.tile([1, BS], F32, tag="hsp")
                        for c in range(NBLK):
                            if c % 3 == 0:
                                nc.scalar.activation(
                                    hT[:, c * BS:(c + 1) * BS],
                                    z[:, c * BS:(c + 1) * BS], Act.Identity,
                                    scale=st[:, c:c + 1],
                                    bias=st[:, NBLK + c:NBLK + c + 1])
                            else:
                                eng = (nc.vector, nc.gpsimd)[c % 2]
                                eng.tensor_scalar(
                                    out=hT[:, c * BS:(c + 1) * BS],
                                    in0=z[:, c * BS:(c + 1) * BS],
                                    scalar1=st[:, c:c + 1],
                                    scalar2=st[:, NBLK + c:NBLK + c + 1],
                                    op0=Alu.mult, op1=Alu.add)
                            hsq = hsqp.tile([128, BS], BF16, tag="hsq")
                            eng2 = (nc.gpsimd, nc.vector)[c % 2]
                            eng2.tensor_tensor(
                                out=hsq[:], in0=hT[:, c * BS:(c + 1) * BS],
                                in1=hT[:, c * BS:(c + 1) * BS], op=Alu.mult)
                            nc.tensor.matmul(hsp[:], onec[:], hsq[:],
                                             start=(c == 0),
                                             stop=(c == NBLK - 1))
                        # e2 rows: [hsq_hi; hsq_lo] (bf16 hi/lo split)
                        hhi = hsqp.tile([1, BS], BF16, tag="hsq")
                        hlo = hsqp.tile([1, BS], BF16, tag="hsq")
                        nc.scalar.copy(hhi[:], hsp[:])
                        nc.vector.tensor_tensor(
                            out=hlo[:], in0=hsp[:], in1=hhi[:],
                            op=Alu.subtract)
                        nc.sync.dma_start(e2t[0:1, :], hhi[:])
                        nc.sync.dma_start(e2t[1:2, :], hlo[:])
                    else:
                        # ---- out^T = s*z + t (bf16), DMA-transpose, +x ----
                        ots = []
                        for c in range(NBLK):
                            ot = otp.tile([128, BS], BF16, tag="ot")
                            eng = (nc.vector, nc.gpsimd)[c % 2]
                            eng.tensor_scalar(
                                out=ot[:], in0=z[:, c * BS:(c + 1) * BS],
                                scalar1=st[:, c:c + 1],
                                scalar2=st[:, NBLK + c:NBLK + c + 1],
                                op0=Alu.mult, op1=Alu.add)
                            ots.append(ot)
                        onats = []
                        for j in range(BS // 128):
                            onat = onatp.tile([128, N], BF16, tag="onat")
                            for c in range(NBLK):
                                nc.scalar.dma_start(
                                    onat[:, c * 128:(c + 1) * 128],
                                    ots[c][:, j * 128:(j + 1) * 128],
                                    transpose=True)
                            onats.append(onat)
                        for j in range(BS // 128):
                            on = outp.tile([128, N], F32, tag="on")
                            eng = (nc.vector, nc.gpsimd)[j % 2]
                            eng.tensor_tensor(out=on[:], in0=onats[j][:],
                                              in1=xnts[j][:], op=Alu.add)
                            nc.sync.dma_start(
                                out[j * 128:(j + 1) * 128, :], on[:])

            for _rep in range(reps):
                body()

    nc.compile()
    return nc


def _host_prep(x, mu1, lam1, v1, g1, b1, mu2, lam2, v2, g2, b2):
    """Build device-input arrays: bf16 pre-tiled weights, per-core x slices."""
    bf16 = ml_dtypes.bfloat16

    def build_w(mu, lam_, v):
        mu64 = mu.astype(np.float64)
        v64 = v.astype(np.float64)
        lam64 = lam_.astype(np.float64)
        Wcols = np.empty((D, NBLK * BCOLS), np.float32)   # weight rows
        wmu = (-2.0 * lam64[:, None] * mu64).T            # (d, n)
        cv = -(v64 * mu64[:, None, :]).sum(-1)            # (n, k)
        for c in range(NBLK):
            ns = slice(c * 128, (c + 1) * 128)
            base = c * BCOLS
            Wcols[:, base:base + 128] = wmu[:, ns]
            for k in range(K):
                b0 = base + (k + 1) * 128
                Wcols[:, b0:b0 + 128] = v[ns, k, :].T
        # tile to SBUF layout: [128, (c ch) * 640]
        Wt = np.empty((128, WTC), np.float32)
        for c in range(NBLK):
            for ch in range(CH):
                Wt[:, (c * CH + ch) * BCOLS:(c * CH + ch + 1) * BCOLS] = \
                    Wcols[ch * 128:(ch + 1) * 128, c * BCOLS:(c + 1) * BCOLS]
        # cv as per-partition f32 columns: [128, 4*NBLK], col = c*4+k
        cvc = cv.reshape(NBLK, 128, K).transpose(1, 0, 2).reshape(128, -1)
        return Wt.astype(bf16), cvc.astype(np.float32)

    W1T, cv1 = build_w(mu1, lam1, v1)
    W2T, cv2 = build_w(mu2, lam2, v2)
    wel_a = np.empty((2, 2 * N), np.float32)
    wel_a[0, 0:N] = lam1
    wel_a[1, 0:N] = lam1
    wel_a[0, N:] = lam2
    wel_a[1, N:] = lam2
    cvt_a = np.concatenate([cv1, cv2], axis=1)
    xT = x.T.astype(np.float32)                           # (d, b)
    xsq = (x.astype(np.float64) ** 2).sum(1).astype(np.float32)
    gbt = np.stack([g1, b1, g2, b2]) \
        .reshape(4, NBLK, 128).transpose(2, 0, 1).reshape(128, 4 * NBLK) \
        .astype(np.float32)
    onesc = np.ones((128, 1), bf16)

    in_maps = []
    for core in range(N_CORES):
        rs = slice(core * BS, (core + 1) * BS)
        xTs = xT[:, rs]                                   # (1024, 512)
        xtt = np.empty((128, CH * BS), np.float32)
        for ch in range(CH):
            xtt[:, ch * BS:(ch + 1) * BS] = xTs[ch * 128:(ch + 1) * 128, :]
        sq = xsq[rs].astype(np.float32)
        sq_hi = sq.astype(bf16)
        sq_lo = (sq.astype(np.float64)
                 - sq_hi.astype(np.float64)).astype(bf16)
        e1 = np.empty((2, BS), bf16)
        e1[0] = sq_hi
        e1[1] = sq_lo
        in_maps.append({
            "xtb": xtt.astype(bf16),
            "xn_s": np.ascontiguousarray(x[rs]).astype(np.float32),
            "e1_s": e1,
            "W1t": W1T, "W2t": W2T,
            "wel": wel_a.astype(bf16), "cvt": cvt_a,
            "gbt": gbt, "onescol": onesc,
        })
    return in_maps


def kernel(x, mu1, lam1, v1, g1, b1, mu2, lam2, v2, g2, b2):
    if "nc" not in _CACHE:
        _CACHE["nc"] = _build_nc()
    nc = _CACHE["nc"]
    in_maps = _host_prep(x, mu1, lam1, v1, g1, b1, mu2, lam2, v2, g2, b2)
    res = run_bass_kernel_spmd(nc, in_maps, list(range(N_CORES)))
    return np.concatenate([res.results[c]["out"] for c in range(N_CORES)], axis=0)
